# revision 1
# baseline (speedup 1.0000x reference)
"""HDCNN (hyperbolic dilated-ish CNN) Trainium2 kernel.

Math (reference): 4 layers of
    v    = out[:, :8192]
    u    = convolve_full(v, w[i])                # [B, 8703], then zero-pad
    hyp  = proj(expmap0(u, c), c)                # c = 1e-12
    out  = relu(mobius_add(hyp, bk_i, c))

Because c = 1e-12, all the Poincare-ball algebra collapses to per-row
scalars that are 1 + O(1e-6).  With per-row scalars folded:

    out_true = g_new * relu(u_stored + eps' * bk)      (elementwise)

where g_new, eps' depend only on n2 = ||u||^2 and p = <u, bk> of the row,
and g_new is carried across layers and applied on the host at the end.
Sensitivity of the output to n2 / p is damped by c, so n2/p are estimated
from every-4th 128-feature chunk (error ~1e-8 relative on the output).

Device layout is transposed: [feature j (partitions), batch b (free)].
Conv is block-Toeplitz matmuls in fp32r (FP22 mantissa, full PE rate).
Sharding: pure data-parallel over batch, 512 rows per core x 8 cores.
"""

import numpy as np

C = 1e-12
FL = 512
IN = 8192
NL = 4
B = 4096
NCORES = 8
NB = B // NCORES          # batch per core (free dim)
NCH_IN = IN // 128        # 64 input chunks
NCH_CONV = 68             # conv output chunks (68*128 = 8704 >= 8703)
NCH_FINAL = 80            # layer-3 output chunks (10240)
SAMPLED = tuple(range(0, 65, 4))   # dot-product chunks; 68/17 == 4 exactly
P_SCALE = float(NCH_CONV // len(SAMPLED))  # 4.0 (dot only; n2 is a full reduction)
MAXT = float(1.0 - 4e-3)           # sqrt(c)*maxnorm of the Poincare projection
BK_OFF = (0, 68, 136, 204)
BK_NQ = (68, 68, 68, 80)
BK_COLS = 284
WT_COLS = NL * 5 * 128 + 128   # trailing 128 cols = ones block

_PROG_CACHE = {}


def _build_program(y2s, repeat=1):
    """Build the per-core Bass program. y2s: list of 4 python floats (||bk_i||^2).

    repeat>1 unrolls the whole computation R times (for timing amplification
    in test harnesses; the result is identical)."""
    import concourse.bacc as bacc
    import concourse.tile as tile
    import concourse.mybir as mybir

    f32 = mybir.dt.float32
    f32r = mybir.dt.float32r
    OP = mybir.AluOpType
    AF = mybir.ActivationFunctionType

    nc = bacc.Bacc("TRN2", target_bir_lowering=False, debug=False)
    hkT = nc.dram_tensor("hkT", [IN, NB], f32r, kind="ExternalInput").ap()
    wt = nc.dram_tensor("wt", [128, WT_COLS], f32r, kind="ExternalInput").ap()
    bkc = nc.dram_tensor("bkc", [128, BK_COLS], f32r, kind="ExternalInput").ap()
    out = nc.dram_tensor("out", [NCH_FINAL * 128, NB], f32, kind="ExternalOutput").ap()
    outg = nc.dram_tensor("outg", [1, NB], f32, kind="ExternalOutput").ap()

    with tile.TileContext(nc) as tc:
        with (
            tc.tile_pool(name="consts", bufs=1) as consts,
            tc.tile_pool(name="upool", bufs=70) as upool,
            tc.tile_pool(name="tpool", bufs=4) as tpool,
            tc.tile_pool(name="qpool", bufs=2) as qpool,
            tc.tile_pool(name="chain", bufs=10) as chain,
            tc.tile_pool(name="long", bufs=2) as longp,
            tc.tile_pool(name="red", bufs=12) as redp,
            tc.tile_pool(name="cpsum", bufs=6, space="PSUM") as cpsum,
            tc.tile_pool(name="apsum", bufs=2, space="PSUM") as apsum,
        ):
            wt_s = consts.tile([128, WT_COLS], f32r, tag="wt")
            nc.sync.dma_start(out=wt_s, in_=wt)
            bkc_s = consts.tile([128, BK_COLS], f32r, tag="bkc")
            nc.sync.dma_start(out=bkc_s, in_=bkc)
            ones = wt_s[:, NL * 5 * 128:NL * 5 * 128 + 128]
            cb1 = consts.tile([128, 1], f32, tag="cb1")
            nc.vector.memset(cb1, 1.0)
            cb2 = consts.tile([128, 1], f32, tag="cb2")
            nc.vector.memset(cb2, 2.0)

            for rep in range(repeat):
                _emit_body(nc, tc, rep, y2s, hkT, out, outg, wt_s, bkc_s, ones, cb1, cb2,
                           upool, tpool, qpool, chain, longp, redp, cpsum, apsum,
                           f32, f32r, OP, AF)

    nc.compile()
    return nc


def _emit_body(nc, tc, rep, y2s, hkT, out, outg, wt_s, bkc_s, ones, cb1, cb2,
               upool, tpool, qpool, chain, longp, redp, cpsum, apsum,
               f32, f32r, OP, AF):
    if True:
        if True:
            # layer-0 inputs: v^T chunks straight from DRAM
            V = []
            for c in range(NCH_IN):
                vtile = upool.tile([128, NB], f32r, tag="u", name=f"r{rep}v0_{c}")
                nc.sync.dma_start(out=vtile, in_=hkT[c * 128:(c + 1) * 128, :])
                V.append(vtile)

            G = None  # carried scale tile; None means 1.0 (layer 0)

            for i in range(NL):
                last = i == NL - 1
                nq_out = NCH_CONV if last else NCH_IN
                cy2 = C * y2s[i]

                acc_n = apsum.tile([1, NB], f32, tag="acc", name=f"accn{i}")
                acc_p = apsum.tile([1, NB], f32, tag="acc", name=f"accp{i}")

                U = {}
                for q in range(NCH_CONV):
                    sampled = q in SAMPLED
                    needed_out = q < nq_out
                    ps = cpsum.tile([128, NB], f32, tag="ps", name=f"ps{i}_{q}")
                    ds = [d for d in range(5) if 0 <= q - d < NCH_IN]
                    for k, d in enumerate(ds):
                        wslice = wt_s[:, (i * 5 + d) * 128:(i * 5 + d + 1) * 128]
                        nc.tensor.matmul(
                            ps,
                            lhsT=wslice,
                            rhs=V[q - d],
                            start=(k == 0),
                            stop=(k == len(ds) - 1),
                        )
                    # full ||u||^2: square every chunk, PE-accumulate over partitions
                    usq = qpool.tile([128, NB], f32r, tag="usq", name=f"usq{i}_{q}")
                    nc.scalar.square(usq, ps)
                    nc.tensor.matmul(
                        acc_n,
                        lhsT=ones[:, 0:1],
                        rhs=usq,
                        start=(q == 0),
                        stop=(q == NCH_CONV - 1),
                    )
                    if sampled or needed_out:
                        u = upool.tile([128, NB], f32r, tag="u", name=f"u{i}_{q}")
                        nc.scalar.copy(u, ps)
                        U[q] = u
                    if sampled:
                        nc.tensor.matmul(
                            acc_p,
                            lhsT=bkc_s[:, BK_OFF[i] + q:BK_OFF[i] + q + 1],
                            rhs=U[q],
                            start=(q == SAMPLED[0]),
                            stop=(q == SAMPLED[-1]),
                        )

                # ---- per-row hyperbolic scalars on [1, NB] rows ----
                # x = sqrt(c * n2_true);  T = min(tanh(x), 1-eps)  (expmap+proj)
                # s_hyp = T/x ;  c*x2 = T^2 ;  Q = 2c*xy = 2c*P_SCALE*acc_p*g*T/x
                # alpha = 1+Q+cy2 ; beta = 1-T^2 ; denom = 1+Q+cy2*T^2
                # A = alpha*g*T/x ; eps' = beta/A ; g_new = A/denom
                if G is None:
                    w2 = redp.tile([1, NB], f32, tag="red", name=f"w2_{i}")
                    nc.scalar.activation(w2, acc_n, AF.Copy, scale=C)
                else:
                    g2 = redp.tile([1, NB], f32, tag="red", name=f"g2_{i}")
                    nc.vector.tensor_tensor(g2, G, G, OP.mult)
                    w2 = redp.tile([1, NB], f32, tag="red", name=f"w2_{i}")
                    nc.vector.scalar_tensor_tensor(w2, acc_n, C, g2, OP.mult, OP.mult)
                x = redp.tile([1, NB], f32, tag="red", name=f"x_{i}")
                nc.scalar.sqrt(x, w2)
                th = redp.tile([1, NB], f32, tag="red", name=f"th_{i}")
                nc.scalar.activation(th, x, AF.Tanh)
                T = redp.tile([1, NB], f32, tag="red", name=f"T_{i}")
                nc.vector.tensor_scalar(T, th, MAXT, None, OP.min)
                zx = redp.tile([1, NB], f32, tag="red", name=f"zx_{i}")
                nc.vector.reciprocal(zx, x)
                H = redp.tile([1, NB], f32, tag="red", name=f"H_{i}")
                nc.vector.tensor_tensor(H, T, zx, OP.mult)
                if G is not None:
                    H2 = redp.tile([1, NB], f32, tag="red", name=f"H2_{i}")
                    nc.vector.tensor_tensor(H2, H, G, OP.mult)
                    H = H2
                Q = redp.tile([1, NB], f32, tag="red", name=f"Q_{i}")
                nc.vector.scalar_tensor_tensor(Q, acc_p, 2.0 * C * P_SCALE, H,
                                               OP.mult, OP.mult)
                X = redp.tile([1, NB], f32, tag="red", name=f"X_{i}")
                nc.vector.tensor_scalar(X, Q, 1.0, None, OP.add)
                T2 = redp.tile([1, NB], f32, tag="red", name=f"T2_{i}")
                nc.vector.tensor_tensor(T2, T, T, OP.mult)
                beta = redp.tile([1, NB], f32, tag="red", name=f"beta_{i}")
                nc.vector.tensor_scalar(beta, T2, -1.0, 1.0, OP.mult, OP.add)
                denom = redp.tile([1, NB], f32, tag="red", name=f"den_{i}")
                nc.vector.scalar_tensor_tensor(denom, T2, cy2, X, OP.mult, OP.add)
                alpha = redp.tile([1, NB], f32, tag="red", name=f"al_{i}")
                nc.vector.tensor_scalar(alpha, X, cy2, None, OP.add)
                A = redp.tile([1, NB], f32, tag="red", name=f"A_{i}")
                nc.vector.tensor_tensor(A, alpha, H, OP.mult)
                rA = redp.tile([1, NB], f32, tag="red", name=f"rA_{i}")
                nc.vector.reciprocal(rA, A)
                rD = redp.tile([1, NB], f32, tag="red", name=f"rD_{i}")
                nc.vector.reciprocal(rD, denom)
                epr = redp.tile([1, NB], f32, tag="red", name=f"epr_{i}")
                nc.vector.tensor_tensor(epr, beta, rA, OP.mult)
                Gn = longp.tile([1, NB], f32, tag="G", name=f"G_{i}")
                nc.vector.tensor_tensor(Gn, A, rD, OP.mult)
                G = Gn

                # replicate eps' across partitions: f32r row -> ones-matmul
                eprr = redp.tile([1, NB], f32r, tag="redr", name=f"eprr_{i}", bufs=2)
                nc.scalar.copy(eprr, epr)
                epp = cpsum.tile([128, NB], f32, tag="ps", name=f"epp_{i}")
                nc.tensor.matmul(epp, lhsT=ones[0:1, :], rhs=eprr,
                                 start=True, stop=True)
                epf = longp.tile([128, NB], f32, tag="epf", name=f"epf_{i}")
                nc.scalar.copy(epf, epp)

                # ---- output phase ----
                Vn = []
                for q in range(nq_out):
                    bcol = bkc_s[:, BK_OFF[i] + q:BK_OFF[i] + q + 1]
                    t = tpool.tile([128, NB], f32, tag="t", name=f"t{i}_{q}")
                    nc.vector.scalar_tensor_tensor(t, epf, bcol.bitcast(f32), U[q].bitcast(f32), OP.mult, OP.add)
                    o = upool.tile([128, NB], f32r, tag="u", name=f"o{i}_{q}")
                    if q % 2 == 0:
                        nc.scalar.activation(o, t, AF.Relu)
                    else:
                        nc.vector.tensor_scalar(o, t, 0.0, None, OP.max)
                    if last:
                        nc.sync.dma_start(out=out[q * 128:(q + 1) * 128, :], in_=o.bitcast(f32))
                    else:
                        Vn.append(o)
                if last:
                    for q in range(NCH_CONV, NCH_FINAL):
                        bcol = bkc_s[:, BK_OFF[i] + q:BK_OFF[i] + q + 1]
                        o = tpool.tile([128, NB], f32, tag="t", name=f"tail{q}")
                        nc.vector.tensor_scalar(o, epf, bcol.bitcast(f32), None, OP.mult)
                        nc.sync.dma_start(out=out[q * 128:(q + 1) * 128, :], in_=o)
                    nc.sync.dma_start(out=outg, in_=G[0:1, :])
                V = Vn


def _host_prep(hk, w, bks):
    hkT = np.ascontiguousarray(hk.T)  # [8192, 4096]

    wt_host = np.zeros((128, WT_COLS), np.float32)
    wt_host[:, NL * 5 * 128:] = 1.0
    r = np.arange(128)[:, None]
    m = np.arange(128)[None, :]
    for i in range(NL):
        for d in range(5):
            idx = 128 * d + m - r
            valid = (idx >= 0) & (idx < FL)
            wt_host[:, (i * 5 + d) * 128:(i * 5 + d + 1) * 128] = np.where(
                valid, w[i][np.clip(idx, 0, FL - 1)], 0.0)

    bkc_host = np.zeros((128, BK_COLS), np.float32)
    for i in range(NL):
        nq = BK_NQ[i]
        bkc_host[:, BK_OFF[i]:BK_OFF[i] + nq] = (
            bks[i][:nq * 128].reshape(nq, 128).T)

    y2s = [float(np.sum(b.astype(np.float64) ** 2)) for b in bks]
    return hkT, wt_host, bkc_host, y2s


def kernel(hk, w, bk0, bk1, bk2, bk3):
    from concourse.bass_utils import run_bass_kernel_spmd

    hk = np.asarray(hk, np.float32)
    w = np.asarray(w, np.float32)
    bks = [np.asarray(b, np.float32) for b in (bk0, bk1, bk2, bk3)]
    hkT, wt_host, bkc_host, y2s = _host_prep(hk, w, bks)

    key = tuple(np.float32(y) for y in y2s)
    if key not in _PROG_CACHE:
        _PROG_CACHE[key] = _build_program(y2s)
    nc = _PROG_CACHE[key]

    in_maps = []
    for k in range(NCORES):
        in_maps.append({
            "hkT": np.ascontiguousarray(hkT[:, k * NB:(k + 1) * NB]),
            "wt": wt_host,
            "bkc": bkc_host,
        })
    res = run_bass_kernel_spmd(nc, in_maps, core_ids=list(range(NCORES)))

    full = np.concatenate([res.results[k]["out"] for k in range(NCORES)], axis=1)
    g = np.concatenate([res.results[k]["outg"][0] for k in range(NCORES)])
    final = (full * g[None, :]).T
    return np.ascontiguousarray(final, np.float32)

